# revision 8
# baseline (speedup 1.0000x reference)
"""GQA attention (B=2, S=2048, DIM=2048, H=16, KVH=4, HD=128, RoPE, causal)
on 8 TRN2 NeuronCores.

Sharding: core c -> batch b = c//4, head-group g = c%4 (q heads 4g..4g+3,
which map exactly to kv head g). Each core computes the partial output
attn_heads @ wo_slice.T  ([S, DIM]); the host sums the 4 partials per batch.

Device layout (everything "transposed", feature-major):
  xT   [DIM, S]   bf16   x[b].T
  wqT  [DIM, 512] bf16   (per-head even/odd-permuted, 1/sqrt(HD)-scaled) wq.T
  wkT  [DIM, 128] bf16   permuted wk.T
  wvT  [DIM, 128] bf16   wv.T (not permuted; v is not roped)
  woT  [512, DIM] bf16   wo[:, cols].T
  cosT [128, S]   bf16   [cos; cos] rope table, frequency-major, duplicated
  sinT [128, S]   bf16   [-sin; sin] sign-folded rope table

The per-head even/odd permutation (rows [0,2,..,126,1,3,..,127]) turns RoPE
pair-interleaving into contiguous half-partitions; q.k dot products are
invariant because q and k are permuted identically.

Attention is computed in transposed score layout: scoresT[k, q] so that
probsT feeds the PV matmul directly (lhsT = v natural layout), attnT falls
out in [hd, q] = exactly the lhsT the output projection needs.

Softmax denominators: full-width prob tiles are pairwise tree-summed on the
DVE (bf16 4x mode), then a single ones-stationary matmul per (head, chunk)
reduces across the 128 key partitions; diagonal partial-width tiles are
accumulated straight into the same PSUM group. This keeps the PE cost of
the denominator at ~1 matmul per chunk instead of 1 per (k-tile, chunk).

x is streamed seq-chunk-major (16 [128,512] tiles per chunk) and the
projections consume it in the same order (K -> V -> Q per chunk) so the PE
starts ~1us in instead of waiting for the whole 8MB of x.
"""

import math
import sys

import numpy as np

try:
    import concourse.bacc as bacc  # noqa: F401
except ImportError:
    sys.path.insert(0, "/opt/trn_rl_repo")

import ml_dtypes
import concourse.bacc as bacc
import concourse.tile as tile
from concourse import mybir
from concourse.bass_utils import run_bass_kernel_spmd

BF16 = mybir.dt.bfloat16
F32 = mybir.dt.float32

B, S, DIM = 2, 2048, 2048
H, KVH, HD = 16, 4, 128
N_CORES = 8
P = 128
D_T = DIM // P      # 16 contraction tiles
NH = H // KVH       # 4 q-heads per core
QC = 512            # q-chunk (matmul moving free dim)
QB = S // QC        # 4 q-chunks
S_T = S // P        # 16 s-tiles / k-tiles

_cached = {}


def _build_nc():
    nc = bacc.Bacc("TRN2", target_bir_lowering=False, debug=False,
                   num_devices=N_CORES)
    xT = nc.dram_tensor("xT", [DIM, S], BF16, kind="ExternalInput").ap()
    wqT = nc.dram_tensor("wqT", [DIM, NH * HD], BF16, kind="ExternalInput").ap()
    wkT = nc.dram_tensor("wkT", [DIM, HD], BF16, kind="ExternalInput").ap()
    wvT = nc.dram_tensor("wvT", [DIM, HD], BF16, kind="ExternalInput").ap()
    woT = nc.dram_tensor("woT", [NH * HD, DIM], BF16, kind="ExternalInput").ap()
    cosT = nc.dram_tensor("cosT", [HD, S], BF16, kind="ExternalInput").ap()
    sinT = nc.dram_tensor("sinT", [HD, S], BF16, kind="ExternalInput").ap()
    out = nc.dram_tensor("out", [S, DIM], BF16, kind="ExternalOutput").ap()

    with tile.TileContext(nc) as tc:
        _build_kernel(tc, xT, wqT, wkT, wvT, woT, cosT, sinT, out)
    nc.compile()
    return nc


def _build_kernel(tc, xT, wqT, wkT, wvT, woT, cosT, sinT, out):
    nc = tc.nc
    Exp = mybir.ActivationFunctionType.Exp

    with (
        tc.tile_pool(name="const", bufs=1) as const,
        tc.tile_pool(name="big", bufs=1) as big,
        tc.tile_pool(name="rtmp", bufs=12) as rtmp,
        tc.tile_pool(name="probs", bufs=9) as probs_pool,
        tc.tile_pool(name="zacc", bufs=8) as zacc_pool,
        tc.tile_pool(name="attn", bufs=6) as attn_pool,
        tc.tile_pool(name="rz", bufs=3) as rz_pool,
        tc.tile_pool(name="osb", bufs=6) as osb_pool,
        tc.tile_pool(name="ps", bufs=5, space="PSUM") as ps_pool,
        tc.tile_pool(name="ps_at", bufs=2, space="PSUM") as ps_at_pool,
        tc.tile_pool(name="ps_z", bufs=1, space="PSUM") as ps_z_pool,
    ):
        # ---- constants ----
        ones = const.tile([P, P], BF16, name="ones")
        nc.vector.memset(ones, 1.0)
        # cos_sb = [cos; cos], sin_sb = [-sin; sin] (host-prepared), so the
        # whole rotation is 3 full-width ops on partition-aligned tiles.
        cos_sb = const.tile([HD, S], BF16, name="cos")
        sin_sb = const.tile([HD, S], BF16, name="sin")

        # ---- weights / activations ----
        # DMA issue order = arrival order (each DMA fans across all 16
        # queues, FIFO): wk/wv first (K/V proj gate the start), then x
        # chunk 0, rope tables, wq, remaining x chunks, wo (needed last).
        wk_sb = big.tile([P, D_T, HD], BF16, name="wk")
        wk_r = wkT.rearrange("(t p) j -> p t j", p=P)
        nc.sync.dma_start(out=wk_sb[:, 0:2, :], in_=wk_r[:, 0:2, :])
        nc.sync.dma_start(out=wk_sb[:, 2:D_T, :], in_=wk_r[:, 2:D_T, :])
        wv_sb = big.tile([P, D_T, HD], BF16, name="wv")
        nc.sync.dma_start(out=wv_sb, in_=wvT.rearrange("(t p) j -> p t j", p=P))

        xt_t = [big.tile([P, S], BF16, name=f"xt{dt}") for dt in range(D_T)]

        def load_x_chunk(sc):
            for dt in range(D_T):
                nc.sync.dma_start(
                    out=xt_t[dt][:, sc * QC:(sc + 1) * QC],
                    in_=xT[dt * P:(dt + 1) * P, sc * QC:(sc + 1) * QC])

        load_x_chunk(0)

        for _src, _dst in ((cosT, cos_sb), (sinT, sin_sb)):
            nc.sync.dma_start(out=_dst, in_=_src)

        wq_sb = big.tile([P, D_T, NH * HD], BF16, name="wq")
        wq_r = wqT.rearrange("(t p) j -> p t j", p=P)
        for hh in range(NH):
            nc.sync.dma_start(
                out=wq_sb[:, :, hh * HD:(hh + 1) * HD],
                in_=wq_r[:, :, hh * HD:(hh + 1) * HD])

        for sc in range(1, QB):
            load_x_chunk(sc)

        wo_sb = big.tile([P, NH, DIM], BF16, name="wo")
        nc.sync.dma_start(out=wo_sb,
                          in_=woT.rearrange("(t p) d -> p t d", p=P))

        xt_tiles = {}
        for dt in range(D_T):
            for sc in range(QB):
                xt_tiles[(dt, sc)] = xt_t[dt][:, sc * QC:(sc + 1) * QC]

        qT = big.tile([P, NH, S], BF16, name="qT")
        kT = big.tile([P, S], BF16, name="kT")
        v_sb = big.tile([P, S_T, HD], BF16, name="v")

        def rope(dst, ps, sc):
            """dst (bf16 [128,512] slice) <- rotate(ps).

            ACT stages ps to bf16 SBUF twice (straight + halves swapped via
            ScalarE partition-shifting copies); DVE then runs three
            full-width 16-bit 2x-mode ops against the sign-folded tables:
            dst = st*[cos;cos] + sw*[-sin;sin]."""
            h = HD // 2
            st = rtmp.tile([P, QC], BF16, name="rst")
            sw = rtmp.tile([P, QC], BF16, name="rsw")
            nc.scalar.copy(out=st, in_=ps)
            nc.scalar.copy(out=sw[0:h, :], in_=ps[h:P, :])
            nc.scalar.copy(out=sw[h:P, :], in_=ps[0:h, :])
            cos_c = cos_sb[:, sc * QC:(sc + 1) * QC]
            sin_c = sin_sb[:, sc * QC:(sc + 1) * QC]
            t0 = rtmp.tile([P, QC], BF16, name="rt")
            t1 = rtmp.tile([P, QC], BF16, name="rt")
            nc.vector.tensor_mul(t0, st, cos_c)
            nc.vector.tensor_mul(t1, sw, sin_c)
            nc.vector.tensor_add(dst, t0, t1)

        # ---- projections, x-chunk-major (follows DMA arrival order) ----
        for sc in range(QB):
            # K projection + rope
            ps = ps_pool.tile([P, QC], F32, name="ps")
            for dt in range(D_T):
                nc.tensor.matmul(ps, lhsT=wk_sb[:, dt, :],
                                 rhs=xt_tiles[(dt, sc)],
                                 start=(dt == 0), stop=(dt == D_T - 1))
            rope(kT[:, sc * QC:(sc + 1) * QC], ps, sc)

            # V projection (natural [s, hd] layout)
            for st in range(4 * sc, 4 * sc + 4):
                ps = ps_pool.tile([P, QC], F32, name="ps")
                for dt in range(D_T):
                    nc.tensor.matmul(
                        ps[:, 0:HD],
                        lhsT=xt_tiles[(dt, sc)][:, (st % 4) * P:(st % 4 + 1) * P],
                        rhs=wv_sb[:, dt, :],
                        start=(dt == 0), stop=(dt == D_T - 1))
                nc.scalar.copy(out=v_sb[:, st, :], in_=ps[:, 0:HD])

            # Q projection + rope
            for hh in range(NH):
                ps = ps_pool.tile([P, QC], F32, name="ps")
                for dt in range(D_T):
                    nc.tensor.matmul(ps, lhsT=wq_sb[:, dt, hh * HD:(hh + 1) * HD],
                                     rhs=xt_tiles[(dt, sc)],
                                     start=(dt == 0), stop=(dt == D_T - 1))
                rope(qT[:, hh, sc * QC:(sc + 1) * QC], ps, sc)

        # ---- attention + output projection, per q-chunk ----
        # Chunks run longest-first so the serial tail is the shortest chunk.
        chunks = [(1536, 512), (1024, 512), (512, 512), (0, 512)]
        for ci, (q0, qw) in enumerate(chunks):
            last_chunk = ci == len(chunks) - 1
            nk = (q0 + qw) // P  # causal k-tiles for this q-chunk
            attn_tiles = {}

            for hh in range(NH):
                at_ps = ps_at_pool.tile([P, qw], F32, name="at")
                full_prs = []   # (tile, off) with off == 0; tree-summed below
                part_prs = []   # diagonal partial-width tiles (off > 0)
                tree = []       # pending tree level (full-width bf16 tiles)
                for k in range(nk):
                    # On diagonal tiles only columns q0+off.. are causally
                    # valid; every stage is right-aligned to [off:qw].
                    off = max(0, k * P - q0)
                    diag = k * P >= q0
                    w = qw - off
                    sc_ps = ps_pool.tile([P, QC], F32, name="ps")
                    nc.tensor.matmul(sc_ps[:, off:qw],
                                     lhsT=kT[:, k * P:(k + 1) * P],
                                     rhs=qT[:, hh, q0 + off:q0 + qw],
                                     start=True, stop=True)
                    pr = probs_pool.tile([P, QC], BF16, name="pr")
                    nc.scalar.activation(out=pr[:, off:qw], in_=sc_ps[:, off:qw],
                                         func=Exp)
                    if diag:  # zero where c' < r
                        nc.gpsimd.affine_select(
                            out=pr[:, off:qw], in_=pr[:, off:qw],
                            compare_op=mybir.AluOpType.is_ge,
                            fill=0.0, base=0, pattern=[[1, w]],
                            channel_multiplier=-1)
                    nc.tensor.matmul(at_ps[:, off:qw], lhsT=v_sb[:, k, :],
                                     rhs=pr[:, off:qw],
                                     start=(k == 0), stop=(k == nk - 1))
                    if off == 0:
                        full_prs.append((pr, 0))
                        # eager balanced tree adds on DVE (bf16 4x mode):
                        # binary-counter merging keeps depth ~log2(F) and
                        # amortizes ~1 add per k-step
                        tree.append((pr, 0))
                        while len(tree) >= 2 and tree[-1][1] == tree[-2][1]:
                            s = zacc_pool.tile([P, QC], BF16, name="zs")
                            nc.vector.tensor_add(s, tree[-2][0], tree[-1][0])
                            tree = tree[:-2] + [(s, tree[-1][1] + 1)]
                    else:
                        part_prs.append((pr, off))
                # finish the tree (merge leftover ranks, lowest first)
                while len(tree) > 1:
                    s = zacc_pool.tile([P, QC], BF16, name="zs")
                    nc.vector.tensor_add(s, tree[-2][0], tree[-1][0])
                    tree = tree[:-2] + [(s, tree[-2][1] + 1)]
                if len(full_prs) > 1:
                    full_prs = [(tree[0][0], 0)]

                # ---- softmax denominator + normalize ----
                # The tree over full tiles finished ~3 k-steps ago (the last
                # 3 k-tiles are diagonal partials), so the PE never waits.
                z_ps = ps_z_pool.tile([P, qw], F32, name="z")
                acc = full_prs[0][0]
                nc.tensor.matmul(z_ps, lhsT=ones, rhs=acc,
                                 start=True, stop=(len(part_prs) == 0))
                for i, (pr, off) in enumerate(part_prs):
                    nc.tensor.matmul(z_ps[:, off:qw], lhsT=ones,
                                     rhs=pr[:, off:qw], start=False,
                                     stop=(i == len(part_prs) - 1))
                rz = rz_pool.tile([P, qw], F32, name="rz")
                nc.vector.reciprocal_approx_fast(out=rz, in_=z_ps)
                a_sb = attn_pool.tile([P, qw], BF16, name="attn")
                nc.vector.tensor_mul(a_sb, at_ps, rz)
                attn_tiles[hh] = a_sb

            for st in range(qw // P):
                row0 = q0 + st * P
                for dc in range(DIM // QC):
                    op_ps = ps_pool.tile([P, QC], F32, name="ps")
                    for j in range(NH):
                        nc.tensor.matmul(
                            op_ps, lhsT=attn_tiles[j][:, st * P:(st + 1) * P],
                            rhs=wo_sb[:, j, dc * QC:(dc + 1) * QC],
                            start=(j == 0), stop=(j == NH - 1))
                    o_sb = osb_pool.tile([P, QC], BF16, name="osb")
                    if last_chunk and (st * 4 + dc) % 2 == 0:
                        nc.scalar.copy(out=o_sb, in_=op_ps)
                    else:
                        nc.vector.tensor_copy(out=o_sb, in_=op_ps)
                    nc.sync.dma_start(
                        out=out[row0:row0 + P, dc * QC:(dc + 1) * QC], in_=o_sb)


def _get_nc():
    if "nc" not in _cached:
        _cached["nc"] = _build_nc()
    return _cached["nc"]


def _prep_in_maps(x, freqs_cis, wq, wk, wv, wo):
    bf = ml_dtypes.bfloat16
    perm = np.concatenate([np.arange(0, HD, 2), np.arange(1, HD, 2)])
    scale = 1.0 / math.sqrt(HD)
    wq_p = (wq.reshape(H, HD, DIM)[:, perm, :] * scale).astype(np.float32)
    wk_p = wk.reshape(KVH, HD, DIM)[:, perm, :]
    cos_h = np.ascontiguousarray(freqs_cis[:, :, 0].T)  # [64, S]
    sin_h = np.ascontiguousarray(freqs_cis[:, :, 1].T)
    cosT = np.concatenate([cos_h, cos_h], axis=0).astype(bf)   # [128, S]
    sinT = np.concatenate([-sin_h, sin_h], axis=0).astype(bf)

    in_maps = []
    for c in range(N_CORES):
        b, g = c // KVH, c % KVH
        hq = slice(NH * g, NH * (g + 1))
        in_maps.append({
            "xT": np.ascontiguousarray(x[b].T).astype(bf),
            "wqT": np.ascontiguousarray(
                wq_p[hq].reshape(NH * HD, DIM).T).astype(bf),
            "wkT": np.ascontiguousarray(wk_p[g].T).astype(bf),
            "wvT": np.ascontiguousarray(wv[g * HD:(g + 1) * HD].T).astype(bf),
            "woT": np.ascontiguousarray(
                wo[:, NH * HD * g:NH * HD * (g + 1)].T).astype(bf),
            "cosT": cosT,
            "sinT": sinT,
        })
    return in_maps


def _reduce_outputs(results):
    out = np.zeros((B, S, DIM), np.float32)
    for c in range(N_CORES):
        out[c // KVH] += results[c]["out"].astype(np.float32)
    return out


def kernel(x, freqs_cis, wq, wk, wv, wo, _trace=False, _trace_kwargs=None):
    nc = _get_nc()
    x, freqs_cis, wq, wk, wv, wo = (
        np.asarray(a, np.float32) for a in (x, freqs_cis, wq, wk, wv, wo))
    in_maps = _prep_in_maps(x, freqs_cis, wq, wk, wv, wo)
    res = run_bass_kernel_spmd(nc, in_maps, core_ids=list(range(N_CORES)),
                               trace=_trace, **(_trace_kwargs or {}))
    out = _reduce_outputs(res.results)
    if _trace:
        _cached["last_exec_time_ns"] = res.exec_time_ns
        _cached["last_results"] = res
    return out
